# revision 32
# baseline (speedup 1.0000x reference)
"""HMM loss kernel for Trainium2 (8 NeuronCores, vocab-parallel).

Problem shapes (hardcoded): B,T,K,LS = 4,8,4,4; PH=B*T*K=128, TL=32,
H=512, V=32000, NS=128.

The dominant device cost is sum-of-exp over the generator logits
[n_act, V]: only span-active tokens need logits (~1370 of 4096), the
vocab axis is sharded over the 8 cores (4000 columns each), and the
matmul runs in fp8 DoubleRow (0.5 cycles/row).  The bottleneck is not
the PE but the exp itself: ScalarE's ACT processes 1 elem/cycle/lane at
1.2 GHz, ~44k columns/core.  This version splits the exp between two
engines:

  - ScalarE: exact exp with fused accumulation on ~60%% of columns.
  - DVE: Schraudolph-style approximate exp on the rest.  Pass 1 is a
    tensor_scalar computing w = round(logit*log2e*128 + (127-c)*128)
    with f32->int16 convert-on-write; the int16 bit pattern IS the bf16
    encoding of 2^(logit*log2e + ...) (sign 0, 8-bit exponent, 7-bit
    mantissa), i.e. a piecewise-linear 2^x with the classic (1+f)/2^f
    sawtooth.  c = log2(E_f[(1+f)/2^f]) centers the error (~+-2%%,
    zero-mean), harmless under the sum.  Pass 2 reinterprets the tile
    as bf16 and runs a 4x-perf-mode tensor_scalar (*1.0) whose fused
    accum_out produces the partial sum-of-exp in f32.

Target logits (psk numerator) are a ~1.4 MFLOP host-side dot; the log,
span sums, chain scores, and the T=8/K=4 backward scan also run on the
host as in the original.
"""

import math
from contextlib import ExitStack

import ml_dtypes
import numpy as np

B, T, K, LS = 4, 8, 4, 4
PH, TL, H, V, NS = B * T * K, 32, 512, 32000, 128
NCORES = 8
VS = V // NCORES  # vocab shard per core
VT_WIDTHS = [512] * (VS // 512) + ([VS % 512] if VS % 512 else [])
HC = H // 128  # contraction chunks
XSCALE = 16.0  # fp8 pre-scales keep W (sigma~0.02) out of e4m3 subnormals
WSCALE = 256.0
LOG2E = math.log2(math.e)
SCHRAUDOLPH_C = 0.0574  # log2 of E_f[(1+f)2^-f]: centers the sawtooth
# PSUM layout per chunk: 4 single-buffered 2-bank groups.  1024-col groups
# keep the per-buffer fill+consume cycle well under the chunk cadence
# (2048-col groups serialize fill->ACT->refill and cost ~1us/chunk).
GROUPS = (1024, 1024, 1024, 928)  # 4000 cols per chunk
# Engine assignment varies per chunk to match DMA arrivals and drain both
# engines simultaneously at the end:
#  - chunks < HEAD_DVE: DVE also takes g1[512:] (it would otherwise idle
#    until the W-g2 DMA lands)
#  - chunks >= ACT_TAKES_G2_FROM: ScalarE consumes all of g2 (the g2 ring
#    becomes purely ACT-paced; DVE keeps only g3)
HEAD_DVE = 0
ACT_TAKES_G2_FROM = 8
WARM = 6  # f32 junk matmuls spinning the PE to full p-state before W lands
STAGGER = 2  # g0/g1 (ScalarE) stream runs this many chunks ahead of g2/g3


def _split_sync_waits(nc, maxw=1):
    """This container's walrus rejects instructions carrying more than a
    couple of sync-wait commands, while Tile freely attaches one wait per
    dependency.  Hoist excess waits onto standalone EventSemaphore
    instructions inserted just before the owner on the same engine queue."""
    import concourse.mybir as mybir

    ctr = 0
    for fn in nc.m.functions:
        for bb in fn.blocks:
            out = []
            changed = False
            for inst in bb.instructions:
                si = getattr(inst, "sync_info", None)
                waits = list(si.on_wait) if si is not None and si.on_wait else []
                if len(waits) > maxw:
                    changed = True
                    extra, keep = waits[:-maxw], waits[-maxw:]
                    for i in range(0, len(extra), maxw):
                        ctr += 1
                        out.append(
                            mybir.InstEventSemaphore(
                                name=f"W-split-{ctr}",
                                engine=inst.engine,
                                ins=[],
                                outs=[],
                                sync_info=mybir.SyncInfo(
                                    on_wait=extra[i : i + maxw], on_update=[]
                                ),
                            )
                        )
                    inst.sync_info = mybir.SyncInfo(
                        on_wait=keep, on_update=list(si.on_update or [])
                    )
                out.append(inst)
            if changed:
                bb.instructions = out


_BUILD_CACHE = {}


def _build(n_chunks, with_bias, repeat=1, head_dve=HEAD_DVE,
           act_g2_from=ACT_TAKES_G2_FROM, stagger=STAGGER, xt_head=3,
           warm=WARM, w_order=(0, 1, 2, 3), act_g3_last=0, dve_g1_last=0):
    """Build the per-core bass program.

    n_chunks: number of 128-token chunks of active tokens.
    head_dve: chunks whose g1[512:] goes to the DVE (early, while W-g2
      is still in flight).
    act_g2_from: chunk index from which ScalarE consumes all of g2.
    stagger: emit chunk i's g0/g1 work alongside chunk i-stagger's g2/g3.
    xt_head: chunks in the first xt DMA (lets PE start early).
    repeat: emit the whole body this many times (timing harness only).
    """
    key = (n_chunks, with_bias, repeat, head_dve, act_g2_from, stagger,
           xt_head, warm, tuple(w_order), act_g3_last, dve_g1_last, 6)
    if key in _BUILD_CACHE:
        return _BUILD_CACHE[key]

    import concourse.bass as bass
    import concourse.mybir as mybir
    import concourse.tile as tile

    f8 = mybir.dt.float8e4
    bf16 = mybir.dt.bfloat16
    i16 = mybir.dt.int16
    f32 = mybir.dt.float32
    exp_scale = float(1.0 / (XSCALE * WSCALE))
    ts_a = LOG2E * 128.0 * exp_scale
    ts_b = (127.0 - SCHRAUDOLPH_C) * 128.0
    goff = [0, 1024, 2048, 3072]  # group column offsets in the vocab shard

    nc = bass.Bass()
    # xt: [partition(h%128), chunk, h//128, token] fp8
    xt_d = nc.dram_tensor("xt", [128, n_chunks, HC, 128], f8, kind="ExternalInput")
    w_d = nc.dram_tensor("w", [128, HC, VS], f8, kind="ExternalInput")
    if with_bias:
        bb_d = nc.dram_tensor("bb", [1, VS], bf16, kind="ExternalInput")
    se_d = nc.dram_tensor("se", [128, n_chunks * 4], f32, kind="ExternalOutput")

    with tile.TileContext(nc) as tc, ExitStack() as ctx:
        consts = ctx.enter_context(tc.tile_pool(name="consts", bufs=2))
        gpools = [
            ctx.enter_context(tc.tile_pool(name=f"ps{g}", bufs=1, space="PSUM"))
            for g in range(4)
        ]
        wpool = ctx.enter_context(tc.tile_pool(name="wpool", bufs=2))
        outp = ctx.enter_context(tc.tile_pool(name="outp", bufs=1))

        for _rep in range(repeat):
            xt_sb = consts.tile([128, n_chunks, HC, 128], f8, tag="xt")
            head = min(xt_head, n_chunks)
            w_sb = [
                consts.tile([128, HC, 1024], f8, tag=f"wg{g}", name=f"wg{g}")
                for g in range(4)
            ]

            def dma_w(g, lo=0, hi=None):
                gw = GROUPS[g] if hi is None else hi
                nc.sync.dma_start(
                    out=w_sb[g][:, :, lo:gw],
                    in_=w_d[:, :, goff[g] + lo : goff[g] + gw],
                )

            # W-g0a goes out on the Pool queue's SWDGE path: it skips the
            # shared HWDGE track, so the transfer gating the first ACT
            # starts ~0.5us earlier.
            nc.gpsimd.dma_start(
                out=w_sb[0][:, :, :512], in_=w_d[:, :, 0:512]
            )
            nc.sync.dma_start(out=xt_sb[:, :head], in_=xt_d[:, :head])
            for g in w_order:
                if g == 0:
                    dma_w(0, 512)
                else:
                    dma_w(g)
            if head < n_chunks:
                nc.sync.dma_start(out=xt_sb[:, head:], in_=xt_d[:, head:])
            if with_bias:
                ones_sb = consts.tile([1, 128], bf16, tag="ones")
                nc.vector.memset(ones_sb, 1.0)
                b_sb = consts.tile([1, VS], bf16, tag="bias")
                nc.sync.dma_start(out=b_sb, in_=bb_d[0:1, :])
            se_all = outp.tile([128, n_chunks * 4], f32, tag="se")
            nc.vector.memset(se_all, 0.0)  # not every slot is written

            if warm:
                junk_sb = consts.tile([128, 128], f32, tag="junk")
                nc.vector.memset(junk_sb, 0.0)
                ps_warm = gpools[3].tile(
                    [128, 1024], f32, tag="g3", name="ps_warm"
                )
                for _ in range(warm):  # spin PE to full p-state before W lands
                    nc.tensor.matmul(
                        ps_warm[:, :128], lhsT=junk_sb, rhs=junk_sb,
                        start=True, stop=True,
                    )

            def fill_into(ps, g, tci, base, wlo=0, whi=None):
                """Matmuls producing vocab group g for chunk tci at column
                `base` of PSUM tile ps."""
                gw = GROUPS[g] if whi is None else whi
                for lo in range(wlo, gw, 512):
                    sw = min(512, gw - lo)
                    pslice = ps[:, base + lo : base + lo + sw]
                    for s in range(0, HC, 2):
                        nc.tensor.matmul(
                            pslice,
                            lhsT=xt_sb[:, tci, s : s + 2, :],
                            rhs=w_sb[g][:, s : s + 2, lo : lo + sw],
                            start=(s == 0),
                            stop=(s == HC - 2) and not with_bias,
                            perf_mode=mybir.MatmulPerfMode.DoubleRow,
                        )
                    if with_bias:
                        voff = goff[g] + lo
                        nc.tensor.matmul(
                            pslice,
                            lhsT=ones_sb,
                            rhs=b_sb[:, voff : voff + sw],
                            start=False,
                            stop=True,
                        )

            def fill(g, tci):
                ps = gpools[g].tile([128, 1024], f32, tag=f"g{g}", name="ps")
                fill_into(ps, g, tci, 0)
                return ps

            def act_exp(ps, cols, slot):
                nc.scalar.activation(
                    out=ps[:, :cols],
                    in_=ps[:, :cols],
                    func=mybir.ActivationFunctionType.Exp,
                    scale=exp_scale,
                    accum_out=se_all[:, slot : slot + 1],
                )

            def schraudolph(wt_slice, ps_slice):
                nc.vector.tensor_scalar(
                    out=wt_slice,
                    in0=ps_slice,
                    scalar1=ts_a,
                    scalar2=ts_b,
                    op0=mybir.AluOpType.mult,
                    op1=mybir.AluOpType.add,
                )

            g1_dve = {}
            for i in range(n_chunks + stagger):
                if i < n_chunks:  # ScalarE stream: g0/g1 of chunk i
                    if i == 0:
                        # consume per-512 so work starts as W-g0a lands
                        ps0 = gpools[0].tile(
                            [128, 1024], f32, tag="g0", name="ps"
                        )
                        fill_into(ps0[:, :512], 0, i, 0, wlo=0, whi=512)
                        act_exp(ps0, 512, i * 4 + 0)
                        fill_into(
                            ps0[:, 512:], 0, i, -512, wlo=512, whi=1024
                        )
                        nc.scalar.activation(
                            out=ps0[:, 512:1024],
                            in_=ps0[:, 512:1024],
                            func=mybir.ActivationFunctionType.Exp,
                            scale=exp_scale,
                            accum_out=se_all[:, i * 4 + 2 : i * 4 + 3],
                        )
                    else:
                        ps0 = fill(0, i)
                        act_exp(ps0, GROUPS[0], i * 4 + 0)
                    ps1 = fill(1, i)
                    if i >= n_chunks - dve_g1_last:
                        g1_dve[i] = ps1  # DVE consumes this g1 instead
                    elif i < head_dve:
                        act_exp(ps1, 512, i * 4 + 1)
                        w1 = wpool.tile([128, 512], i16, tag="schr1")
                        schraudolph(w1, ps1[:, 512:1024])
                        w1b = w1.bitcast(bf16)
                        nc.vector.tensor_scalar(
                            out=w1b,
                            in0=w1b,
                            scalar1=1.0,
                            scalar2=0.0,
                            op0=mybir.AluOpType.mult,
                            op1=mybir.AluOpType.add,
                            accum_out=se_all[:, i * 4 + 2 : i * 4 + 3],
                        )
                    else:
                        act_exp(ps1, GROUPS[1], i * 4 + 1)
                if i >= stagger:  # DVE stream: g2/g3 of chunk i-stagger
                    tci = i - stagger
                    dve_g2 = 0 if tci >= act_g2_from else 1024
                    dve_g1 = 1024 if tci in g1_dve else 0
                    g3_act = tci >= n_chunks - act_g3_last
                    if dve_g2 == 0 and tci > act_g2_from:
                        # ACT-owned late g2: rotate through the winding-down
                        # g0/g1 pools so fills don't serialize on one buffer
                        lpool = gpools[(tci - act_g2_from - 1) % 2]
                        ps2 = lpool.tile(
                            [128, 1024], f32, tag=f"g{(tci - act_g2_from - 1) % 2}",
                            name="ps",
                        )
                        fill_into(ps2, 2, tci, 0)
                    else:
                        ps2 = fill(2, tci)
                    if dve_g2 == 0:
                        act_exp(ps2, 1024, tci * 4 + 2)
                    dve_w = dve_g1 + dve_g2 + (0 if g3_act else GROUPS[3])
                    wt = None
                    if dve_w:
                        wt = wpool.tile(
                            [128, dve_w], i16, tag="schr", name="wt"
                        )
                    if dve_g1:
                        schraudolph(wt[:, :dve_g1], g1_dve.pop(tci))
                    if dve_g2:
                        schraudolph(wt[:, dve_g1 : dve_g1 + dve_g2], ps2)
                    ps3 = fill(3, tci)
                    if g3_act:
                        act_exp(ps3, GROUPS[3], tci * 4 + 3)
                    else:
                        schraudolph(
                            wt[:, dve_g1 + dve_g2 :], ps3[:, : GROUPS[3]]
                        )
                    if wt is None:
                        continue
                    wb = wt.bitcast(bf16)
                    nc.vector.tensor_scalar(
                        out=wb,
                        in0=wb,
                        scalar1=1.0,
                        scalar2=0.0,
                        op0=mybir.AluOpType.mult,
                        op1=mybir.AluOpType.add,
                        accum_out=se_all[:, tci * 4 + 3 : tci * 4 + 4],
                    )

            nc.sync.dma_start(out=se_d[:, :], in_=se_all)

    _split_sync_waits(nc)
    _BUILD_CACHE[key] = nc
    return nc


def _prep_inputs(output, W, b, target, tgt_idx):
    """Host-side sharding/layout prep. Returns (in_maps, meta)."""
    x = np.asarray(output, np.float32).reshape(PH * TL, H)
    tgt = np.asarray(target, np.int32).reshape(-1)
    ti = np.asarray(tgt_idx, np.int32)
    bv = np.asarray(b, np.float32).reshape(-1)
    with_bias = bool(np.any(bv != 0.0))

    pos = np.arange(TL)
    span = (pos[None, :] >= ti[:, :1]) & (pos[None, :] <= ti[:, 1:2])
    act = np.flatnonzero(span.reshape(-1))
    n_act = int(act.size)
    n_chunks = max(1, math.ceil(n_act / 128))
    n_pad = n_chunks * 128
    act_pad = np.zeros(n_pad, np.int64)
    act_pad[:n_act] = act

    Wf = np.asarray(W, np.float32)
    xa = x[act_pad]  # [n_pad, H] f32

    mmnp = ml_dtypes.float8_e4m3
    x_m = (xa * XSCALE).astype(mmnp)
    w_m = (Wf * WSCALE).astype(mmnp)

    # [n_pad, H] -> [H, n_pad] -> [HC,128,nch,128] -> [128,nch,HC,128]
    xt = np.ascontiguousarray(
        x_m.T.reshape(HC, 128, n_chunks, 128).transpose(1, 2, 0, 3)
    )

    # target logits on host (f32 data, f64 accumulate): ~1.4 MFLOP
    tl_tok = np.einsum(
        "th,th->t", xa.astype(np.float64), Wf.T[tgt[act_pad]].astype(np.float64)
    )

    in_maps = []
    for i in range(NCORES):
        wsh = np.ascontiguousarray(
            w_m[:, i * VS : (i + 1) * VS].reshape(HC, 128, VS).transpose(1, 0, 2)
        )
        m = {"xt": xt, "w": wsh}
        if with_bias:
            m["bb"] = (bv[i * VS : (i + 1) * VS] * (XSCALE * WSCALE)).astype(
                ml_dtypes.bfloat16
            ).reshape(1, VS)
        in_maps.append(m)

    meta = dict(
        act=act, act_pad=act_pad, n_act=n_act, n_chunks=n_chunks, n_pad=n_pad,
        tgt=tgt, with_bias=with_bias, bv=bv, tl_tok=tl_tok,
    )
    return in_maps, meta


def _combine(results, meta):
    """Host-side unshard: total sum-exp across vocab shards -> psk."""
    n_act, n_chunks = meta["n_act"], meta["n_chunks"]
    se = np.zeros((128, n_chunks * 4), np.float64)
    for r in results:
        se += r["se"].astype(np.float64)
    # token t = chunk*128 + lane; four partials per chunk
    sumexp_tok = se.reshape(128, n_chunks, 4).sum(axis=2).T.reshape(-1)
    tl_tok = meta["tl_tok"].copy()
    if meta["with_bias"]:
        tl_tok = tl_tok + meta["bv"][meta["tgt"][meta["act_pad"]]]

    logz = np.log(sumexp_tok[:n_act])
    psk_act = tl_tok[:n_act] - logz
    psk = np.zeros(PH * TL, np.float64)
    psk[meta["act"]] = psk_act
    return psk.reshape(PH, TL)


def _hmm_tail(psk, tgt_idx, states, init_logps, trans_logps, ext_logps, hsmm_sid):
    """Direct numpy port of the reference below the log-softmax."""
    ti = np.asarray(tgt_idx, np.int32)
    st4 = np.asarray(states, np.int64)
    init_logps = np.asarray(init_logps, np.float64)
    trans_logps = np.asarray(trans_logps, np.float64)
    ext_logps = np.asarray(ext_logps, np.float64)
    sid = int(np.asarray(hsmm_sid))

    pos = np.arange(TL)
    span = (pos[None, :] >= ti[:, :1]) & (pos[None, :] <= ti[:, 1:2])
    fwd_obs = np.where(span, psk, 0.0).sum(axis=1)  # [PH]

    st = st4.reshape(PH, LS)
    chain = trans_logps[st[:, :-1], st[:, 1:]].sum(axis=1)  # [PH]
    init_pmt = (init_logps[st[:, 0]] + chain).reshape(B, T, K)
    pmt = chain.reshape(B, T, K)
    obs = fwd_obs.reshape(B, T, K)
    z = np.where((np.arange(T) == 0)[None, :, None], init_pmt, pmt)
    s_first = st4[..., 0]  # [B,T,K]
    s_last = st4[..., -1]
    ov = np.any(
        st4[:, :-1, :, None, :, None] == st4[:, 1:, None, :, None, :], axis=(-1, -2)
    )  # [B,T-1,K,K]

    def lse2(x):  # logsumexp over last axis, -inf safe
        m = np.max(x, axis=-1, keepdims=True)
        ms = np.where(np.isfinite(m), m, 0.0)
        with np.errstate(divide="ignore"):
            return np.log(np.exp(x - ms).sum(axis=-1)) + ms[..., 0]

    beta = np.zeros((B, K), np.float64)
    for t in range(T - 2, -1, -1):
        sl = s_last[:, t]
        sf = s_first[:, t + 1]
        tr = (
            trans_logps[sl[:, :, None], sf[:, None, :]]
            + ext_logps[sl[:, :, None], sf[:, None, :]]
        )
        score = (
            beta[:, None, :]
            + obs[:, t + 1][:, None, :]
            + z[:, t + 1][:, None, :]
            + z[:, t][:, :, None]
            + tr
        )
        if K > 1:
            score = np.where(ov[:, t], -np.inf, score)
        beta = lse2(score)

    score0 = beta + obs[:, 0] + z[:, 0] + ext_logps[sid, s_first[:, 0]]
    log_marg = lse2(score0)
    return -np.sum(log_marg)


def kernel(output, W, b, target, tgt_idx, states, init_logps, trans_logps,
           ext_logps, hsmm_sid):
    from concourse.bass_utils import run_bass_kernel_spmd

    in_maps, meta = _prep_inputs(output, W, b, target, tgt_idx)
    nch = meta["n_chunks"]
    nc = _build(nch, meta["with_bias"], act_g2_from=max(1, nch - 3))
    last_err = None
    for _attempt in range(3):
        try:
            res = run_bass_kernel_spmd(nc, in_maps, core_ids=list(range(NCORES)))
            break
        except Exception as e:  # rare transient device-unrecoverable flakes
            last_err = e
            import time as _time

            _time.sleep(2.0)
    else:
        raise last_err
    psk = _combine(res.results, meta)
    loss = _hmm_tail(psk, tgt_idx, states, init_logps, trans_logps, ext_logps, hsmm_sid)
    return np.float32(loss)


# revision 36
# speedup vs baseline: 1.3274x; 1.3274x over previous
"""HMM loss kernel for Trainium2 (8 NeuronCores, vocab-parallel).

Problem shapes (hardcoded): B,T,K,LS = 4,8,4,4; PH=B*T*K=128, TL=32,
H=512, V=32000, NS=128.

The dominant device cost is sum-of-exp over the generator logits
[n_act, V]: only span-active tokens need logits (~1370 of 4096), the
vocab axis is sharded over the 8 cores (4000 columns each), and the
matmul runs in fp8 DoubleRow (0.5 cycles/row).  The bottleneck is not
the PE but the exp itself: ScalarE's ACT processes 1 elem/cycle/lane at
1.2 GHz, ~44k columns/core.  This version splits the exp between two
engines:

  - ScalarE: exact exp with fused accumulation on ~60%% of columns.
  - DVE: Schraudolph-style approximate exp on the rest.  Pass 1 is a
    tensor_scalar computing w = round(logit*log2e*128 + (127-c)*128)
    with f32->int16 convert-on-write; the int16 bit pattern IS the bf16
    encoding of 2^(logit*log2e + ...) (sign 0, 8-bit exponent, 7-bit
    mantissa), i.e. a piecewise-linear 2^x with the classic (1+f)/2^f
    sawtooth.  c = log2(E_f[(1+f)/2^f]) centers the error (~+-2%%,
    zero-mean), harmless under the sum.  Pass 2 reinterprets the tile
    as bf16 and runs a 4x-perf-mode tensor_scalar (*1.0) whose fused
    accum_out produces the partial sum-of-exp in f32.

Target logits (psk numerator) are a ~1.4 MFLOP host-side dot; the log,
span sums, chain scores, and the T=8/K=4 backward scan also run on the
host as in the original.
"""

import math
from contextlib import ExitStack

import ml_dtypes
import numpy as np

B, T, K, LS = 4, 8, 4, 4
PH, TL, H, V, NS = B * T * K, 32, 512, 32000, 128
NCORES = 8
VS = V // NCORES  # vocab shard per core
VT_WIDTHS = [512] * (VS // 512) + ([VS % 512] if VS % 512 else [])
HC = H // 128  # contraction chunks
XSCALE = 16.0  # fp8 pre-scales keep W (sigma~0.02) out of e4m3 subnormals
WSCALE = 256.0
LOG2E = math.log2(math.e)
SCHRAUDOLPH_C = 0.0574  # log2 of E_f[(1+f)2^-f]: centers the sawtooth
# PSUM layout per chunk: 4 single-buffered 2-bank groups.  1024-col groups
# keep the per-buffer fill+consume cycle well under the chunk cadence
# (2048-col groups serialize fill->ACT->refill and cost ~1us/chunk).
GROUPS = (1024, 1024, 1024, 928)  # 4000 cols per chunk
# Engine assignment varies per chunk to match DMA arrivals and drain both
# engines simultaneously at the end:
#  - chunks < HEAD_DVE: DVE also takes g1[512:] (it would otherwise idle
#    until the W-g2 DMA lands)
#  - chunks >= ACT_TAKES_G2_FROM: ScalarE consumes all of g2 (the g2 ring
#    becomes purely ACT-paced; DVE keeps only g3)
HEAD_DVE = 0
ACT_TAKES_G2_FROM = 8
WARM = 6  # f32 junk matmuls spinning the PE to full p-state before W lands
STAGGER = 2  # g0/g1 (ScalarE) stream runs this many chunks ahead of g2/g3


def _split_sync_waits(nc, maxw=1):
    """This container's walrus rejects instructions carrying more than a
    couple of sync-wait commands, while Tile freely attaches one wait per
    dependency.  Hoist excess waits onto standalone EventSemaphore
    instructions inserted just before the owner on the same engine queue."""
    import concourse.mybir as mybir

    ctr = 0
    for fn in nc.m.functions:
        for bb in fn.blocks:
            out = []
            changed = False
            for inst in bb.instructions:
                si = getattr(inst, "sync_info", None)
                waits = list(si.on_wait) if si is not None and si.on_wait else []
                if len(waits) > maxw:
                    changed = True
                    extra, keep = waits[:-maxw], waits[-maxw:]
                    for i in range(0, len(extra), maxw):
                        ctr += 1
                        out.append(
                            mybir.InstEventSemaphore(
                                name=f"W-split-{ctr}",
                                engine=inst.engine,
                                ins=[],
                                outs=[],
                                sync_info=mybir.SyncInfo(
                                    on_wait=extra[i : i + maxw], on_update=[]
                                ),
                            )
                        )
                    inst.sync_info = mybir.SyncInfo(
                        on_wait=keep, on_update=list(si.on_update or [])
                    )
                out.append(inst)
            if changed:
                bb.instructions = out


_BUILD_CACHE = {}


def _build(n_chunks, with_bias, repeat=1, head_dve=HEAD_DVE,
           act_g2_from=ACT_TAKES_G2_FROM, stagger=STAGGER, xt_head=3,
           warm=WARM, w_order=(0, 1, 2, 3), act_g3_last=0, dve_g1_last=0):
    """Build the per-core bass program.

    n_chunks: number of 128-token chunks of active tokens.
    head_dve: chunks whose g1[512:] goes to the DVE (early, while W-g2
      is still in flight).
    act_g2_from: chunk index from which ScalarE consumes all of g2.
    stagger: emit chunk i's g0/g1 work alongside chunk i-stagger's g2/g3.
    xt_head: chunks in the first xt DMA (lets PE start early).
    repeat: emit the whole body this many times (timing harness only).
    """
    key = (n_chunks, with_bias, repeat, head_dve, act_g2_from, stagger,
           xt_head, warm, tuple(w_order), act_g3_last, dve_g1_last, 6)
    if key in _BUILD_CACHE:
        return _BUILD_CACHE[key]

    import concourse.bass as bass
    import concourse.mybir as mybir
    import concourse.tile as tile

    f8 = mybir.dt.float8e4
    bf16 = mybir.dt.bfloat16
    i16 = mybir.dt.int16
    f32 = mybir.dt.float32
    exp_scale = float(1.0 / (XSCALE * WSCALE))
    ts_a = LOG2E * 128.0 * exp_scale
    ts_b = (127.0 - SCHRAUDOLPH_C) * 128.0
    goff = [0, 1024, 2048, 3072]  # group column offsets in the vocab shard

    nc = bass.Bass()
    # xt: [partition(h%128), chunk, h//128, token] fp8
    xt_d = nc.dram_tensor("xt", [128, n_chunks, HC, 128], f8, kind="ExternalInput")
    w_d = nc.dram_tensor("w", [128, HC, VS], f8, kind="ExternalInput")
    if with_bias:
        bb_d = nc.dram_tensor("bb", [1, VS], bf16, kind="ExternalInput")
    se_d = nc.dram_tensor("se", [128, 4, n_chunks], f32, kind="ExternalOutput")

    with tile.TileContext(nc) as tc, ExitStack() as ctx:
        consts = ctx.enter_context(tc.tile_pool(name="consts", bufs=2))
        gpools = [
            ctx.enter_context(tc.tile_pool(name=f"ps{g}", bufs=1, space="PSUM"))
            for g in range(4)
        ]
        wpool = ctx.enter_context(tc.tile_pool(name="wpool", bufs=2))
        outp = ctx.enter_context(tc.tile_pool(name="outp", bufs=1))

        for _rep in range(repeat):
            xt_sb = consts.tile([128, n_chunks, HC, 128], f8, tag="xt")
            head = min(xt_head, n_chunks)
            w_sb = [
                consts.tile([128, HC, 1024], f8, tag=f"wg{g}", name=f"wg{g}")
                for g in range(4)
            ]

            def dma_w(g, lo=0, hi=None):
                gw = GROUPS[g] if hi is None else hi
                nc.sync.dma_start(
                    out=w_sb[g][:, :, lo:gw],
                    in_=w_d[:, :, goff[g] + lo : goff[g] + gw],
                )

            # W-g0a goes out on the Pool queue's SWDGE path: it skips the
            # shared HWDGE track, so the transfer gating the first ACT
            # starts ~0.5us earlier.
            nc.gpsimd.dma_start(
                out=w_sb[0][:, :, :512], in_=w_d[:, :, 0:512]
            )
            nc.sync.dma_start(out=xt_sb[:, :head], in_=xt_d[:, :head])
            for g in w_order:
                if g == 0:
                    dma_w(0, 512)
                else:
                    dma_w(g)
            if head < n_chunks:
                nc.sync.dma_start(out=xt_sb[:, head:], in_=xt_d[:, head:])
            if with_bias:
                ones_sb = consts.tile([1, 128], bf16, tag="ones")
                nc.vector.memset(ones_sb, 1.0)
                b_sb = consts.tile([1, VS], bf16, tag="bias")
                nc.sync.dma_start(out=b_sb, in_=bb_d[0:1, :])
            se_all = outp.tile([128, 4, n_chunks], f32, tag="se")
            nc.vector.memset(se_all, 0.0)  # not every slot is written

            if warm:
                junk_sb = consts.tile([128, 128], f32, tag="junk")
                nc.vector.memset(junk_sb, 0.0)
                ps_warm = gpools[3].tile(
                    [128, 1024], f32, tag="g3", name="ps_warm"
                )
                for _ in range(warm):  # spin PE to full p-state before W lands
                    nc.tensor.matmul(
                        ps_warm[:, :128], lhsT=junk_sb, rhs=junk_sb,
                        start=True, stop=True,
                    )

            def fill_into(ps, g, tci, base, wlo=0, whi=None):
                """Matmuls producing vocab group g for chunk tci at column
                `base` of PSUM tile ps."""
                gw = GROUPS[g] if whi is None else whi
                for lo in range(wlo, gw, 512):
                    sw = min(512, gw - lo)
                    pslice = ps[:, base + lo : base + lo + sw]
                    for s in range(0, HC, 2):
                        nc.tensor.matmul(
                            pslice,
                            lhsT=xt_sb[:, tci, s : s + 2, :],
                            rhs=w_sb[g][:, s : s + 2, lo : lo + sw],
                            start=(s == 0),
                            stop=(s == HC - 2) and not with_bias,
                            perf_mode=mybir.MatmulPerfMode.DoubleRow,
                        )
                    if with_bias:
                        voff = goff[g] + lo
                        nc.tensor.matmul(
                            pslice,
                            lhsT=ones_sb,
                            rhs=b_sb[:, voff : voff + sw],
                            start=False,
                            stop=True,
                        )

            def fill(g, tci):
                ps = gpools[g].tile([128, 1024], f32, tag=f"g{g}", name="ps")
                fill_into(ps, g, tci, 0)
                return ps

            def act_exp(ps, cols, k, c):
                nc.scalar.activation(
                    out=ps[:, :cols],
                    in_=ps[:, :cols],
                    func=mybir.ActivationFunctionType.Exp,
                    scale=exp_scale,
                    accum_out=se_all[:, k : k + 1, c : c + 1].squeeze(1),
                )

            def schraudolph(wt_slice, ps_slice):
                nc.vector.tensor_scalar(
                    out=wt_slice,
                    in0=ps_slice,
                    scalar1=ts_a,
                    scalar2=ts_b,
                    op0=mybir.AluOpType.mult,
                    op1=mybir.AluOpType.add,
                )

            g1_dve = {}
            for i in range(n_chunks + stagger):
                if i < n_chunks:  # ScalarE stream: g0/g1 of chunk i
                    if i == 0:
                        # consume per-512 so work starts as W-g0a lands
                        ps0 = gpools[0].tile(
                            [128, 1024], f32, tag="g0", name="ps"
                        )
                        fill_into(ps0[:, :512], 0, i, 0, wlo=0, whi=512)
                        act_exp(ps0, 512, 0, i)
                        fill_into(
                            ps0[:, 512:], 0, i, -512, wlo=512, whi=1024
                        )
                        nc.scalar.activation(
                            out=ps0[:, 512:1024],
                            in_=ps0[:, 512:1024],
                            func=mybir.ActivationFunctionType.Exp,
                            scale=exp_scale,
                            accum_out=se_all[:, 2:3, i : i + 1].squeeze(1),
                        )
                    else:
                        ps0 = fill(0, i)
                        act_exp(ps0, GROUPS[0], 0, i)
                    ps1 = fill(1, i)
                    if i >= n_chunks - dve_g1_last:
                        g1_dve[i] = ps1  # DVE consumes this g1 instead
                    elif i < head_dve:
                        act_exp(ps1, 512, 1, i)
                        w1 = wpool.tile([128, 512], i16, tag="schr1")
                        schraudolph(w1, ps1[:, 512:1024])
                        w1b = w1.bitcast(bf16)
                        nc.vector.tensor_scalar(
                            out=w1b,
                            in0=w1b,
                            scalar1=1.0,
                            scalar2=0.0,
                            op0=mybir.AluOpType.mult,
                            op1=mybir.AluOpType.add,
                            accum_out=se_all[:, 2:3, i : i + 1].squeeze(1),
                        )
                    else:
                        act_exp(ps1, GROUPS[1], 1, i)
                if i >= stagger:  # DVE stream: g2/g3 of chunk i-stagger
                    tci = i - stagger
                    dve_g2 = 0 if tci >= act_g2_from else 1024
                    dve_g1 = 1024 if tci in g1_dve else 0
                    g3_act = tci >= n_chunks - act_g3_last
                    if dve_g2 == 0 and tci > act_g2_from:
                        # ACT-owned late g2: rotate through the winding-down
                        # g0/g1 pools so fills don't serialize on one buffer
                        lpool = gpools[(tci - act_g2_from - 1) % 2]
                        ps2 = lpool.tile(
                            [128, 1024], f32, tag=f"g{(tci - act_g2_from - 1) % 2}",
                            name="ps",
                        )
                        fill_into(ps2, 2, tci, 0)
                    else:
                        ps2 = fill(2, tci)
                    if dve_g2 == 0:
                        act_exp(ps2, 1024, 2, tci)
                    dve_w = dve_g1 + dve_g2 + (0 if g3_act else GROUPS[3])
                    wt = None
                    if dve_w:
                        wt = wpool.tile(
                            [128, dve_w], i16, tag="schr", name="wt"
                        )
                    if dve_g1:
                        schraudolph(wt[:, :dve_g1], g1_dve.pop(tci))
                    if dve_g2:
                        schraudolph(wt[:, dve_g1 : dve_g1 + dve_g2], ps2)
                    ps3 = fill(3, tci)
                    if g3_act:
                        act_exp(ps3, GROUPS[3], 3, tci)
                    else:
                        schraudolph(
                            wt[:, dve_g1 + dve_g2 :], ps3[:, : GROUPS[3]]
                        )
                    if wt is None:
                        continue
                    wb = wt.bitcast(bf16)
                    nc.vector.tensor_scalar(
                        out=wb,
                        in0=wb,
                        scalar1=1.0,
                        scalar2=0.0,
                        op0=mybir.AluOpType.mult,
                        op1=mybir.AluOpType.add,
                        accum_out=se_all[:, 3:4, tci : tci + 1].squeeze(1),
                    )

            # ACT-written slots ship from the ACT queue: same-queue order
            # makes the DMA dispatch right after the last accum, skipping
            # the cross-engine semaphore collection on SP.
            nc.sync.dma_start(out=se_d[:, 3:4], in_=se_all[:, 3:4])
            nc.scalar.dma_start(out=se_d[:, 0:3], in_=se_all[:, 0:3])

    _split_sync_waits(nc)
    _BUILD_CACHE[key] = nc
    return nc


def _prep_inputs(output, W, b, target, tgt_idx):
    """Host-side sharding/layout prep. Returns (in_maps, meta)."""
    x = np.asarray(output, np.float32).reshape(PH * TL, H)
    tgt = np.asarray(target, np.int32).reshape(-1)
    ti = np.asarray(tgt_idx, np.int32)
    bv = np.asarray(b, np.float32).reshape(-1)
    with_bias = bool(np.any(bv != 0.0))

    pos = np.arange(TL)
    span = (pos[None, :] >= ti[:, :1]) & (pos[None, :] <= ti[:, 1:2])
    act = np.flatnonzero(span.reshape(-1))
    n_act = int(act.size)
    n_chunks = max(1, math.ceil(n_act / 128))
    n_pad = n_chunks * 128
    act_pad = np.zeros(n_pad, np.int64)
    act_pad[:n_act] = act

    Wf = np.asarray(W, np.float32)
    xa = x[act_pad]  # [n_pad, H] f32

    mmnp = ml_dtypes.float8_e4m3
    x_m = (xa * XSCALE).astype(mmnp)
    w_m = (Wf * WSCALE).astype(mmnp)

    # [n_pad, H] -> [H, n_pad] -> [HC,128,nch,128] -> [128,nch,HC,128]
    xt = np.ascontiguousarray(
        x_m.T.reshape(HC, 128, n_chunks, 128).transpose(1, 2, 0, 3)
    )

    # target logits on host (f32 data, f64 accumulate): ~1.4 MFLOP
    tl_tok = np.einsum(
        "th,th->t", xa.astype(np.float64), Wf.T[tgt[act_pad]].astype(np.float64)
    )

    in_maps = []
    for i in range(NCORES):
        wsh = np.ascontiguousarray(
            w_m[:, i * VS : (i + 1) * VS].reshape(HC, 128, VS).transpose(1, 0, 2)
        )
        m = {"xt": xt, "w": wsh}
        if with_bias:
            m["bb"] = (bv[i * VS : (i + 1) * VS] * (XSCALE * WSCALE)).astype(
                ml_dtypes.bfloat16
            ).reshape(1, VS)
        in_maps.append(m)

    meta = dict(
        act=act, act_pad=act_pad, n_act=n_act, n_chunks=n_chunks, n_pad=n_pad,
        tgt=tgt, with_bias=with_bias, bv=bv, tl_tok=tl_tok,
    )
    return in_maps, meta


def _combine(results, meta):
    """Host-side unshard: total sum-exp across vocab shards -> psk."""
    n_act, n_chunks = meta["n_act"], meta["n_chunks"]
    se = np.zeros((128, 4, n_chunks), np.float64)
    for r in results:
        se += r["se"].astype(np.float64)
    # token t = chunk*128 + lane; four partials per chunk (slot-major)
    sumexp_tok = se.sum(axis=1).T.reshape(-1)
    tl_tok = meta["tl_tok"].copy()
    if meta["with_bias"]:
        tl_tok = tl_tok + meta["bv"][meta["tgt"][meta["act_pad"]]]

    logz = np.log(sumexp_tok[:n_act])
    psk_act = tl_tok[:n_act] - logz
    psk = np.zeros(PH * TL, np.float64)
    psk[meta["act"]] = psk_act
    return psk.reshape(PH, TL)


def _hmm_tail(psk, tgt_idx, states, init_logps, trans_logps, ext_logps, hsmm_sid):
    """Direct numpy port of the reference below the log-softmax."""
    ti = np.asarray(tgt_idx, np.int32)
    st4 = np.asarray(states, np.int64)
    init_logps = np.asarray(init_logps, np.float64)
    trans_logps = np.asarray(trans_logps, np.float64)
    ext_logps = np.asarray(ext_logps, np.float64)
    sid = int(np.asarray(hsmm_sid))

    pos = np.arange(TL)
    span = (pos[None, :] >= ti[:, :1]) & (pos[None, :] <= ti[:, 1:2])
    fwd_obs = np.where(span, psk, 0.0).sum(axis=1)  # [PH]

    st = st4.reshape(PH, LS)
    chain = trans_logps[st[:, :-1], st[:, 1:]].sum(axis=1)  # [PH]
    init_pmt = (init_logps[st[:, 0]] + chain).reshape(B, T, K)
    pmt = chain.reshape(B, T, K)
    obs = fwd_obs.reshape(B, T, K)
    z = np.where((np.arange(T) == 0)[None, :, None], init_pmt, pmt)
    s_first = st4[..., 0]  # [B,T,K]
    s_last = st4[..., -1]
    ov = np.any(
        st4[:, :-1, :, None, :, None] == st4[:, 1:, None, :, None, :], axis=(-1, -2)
    )  # [B,T-1,K,K]

    def lse2(x):  # logsumexp over last axis, -inf safe
        m = np.max(x, axis=-1, keepdims=True)
        ms = np.where(np.isfinite(m), m, 0.0)
        with np.errstate(divide="ignore"):
            return np.log(np.exp(x - ms).sum(axis=-1)) + ms[..., 0]

    beta = np.zeros((B, K), np.float64)
    for t in range(T - 2, -1, -1):
        sl = s_last[:, t]
        sf = s_first[:, t + 1]
        tr = (
            trans_logps[sl[:, :, None], sf[:, None, :]]
            + ext_logps[sl[:, :, None], sf[:, None, :]]
        )
        score = (
            beta[:, None, :]
            + obs[:, t + 1][:, None, :]
            + z[:, t + 1][:, None, :]
            + z[:, t][:, :, None]
            + tr
        )
        if K > 1:
            score = np.where(ov[:, t], -np.inf, score)
        beta = lse2(score)

    score0 = beta + obs[:, 0] + z[:, 0] + ext_logps[sid, s_first[:, 0]]
    log_marg = lse2(score0)
    return -np.sum(log_marg)


def kernel(output, W, b, target, tgt_idx, states, init_logps, trans_logps,
           ext_logps, hsmm_sid):
    from concourse.bass_utils import run_bass_kernel_spmd

    in_maps, meta = _prep_inputs(output, W, b, target, tgt_idx)
    nch = meta["n_chunks"]
    nc = _build(nch, meta["with_bias"], act_g2_from=max(1, nch - 3))
    last_err = None
    for _attempt in range(3):
        try:
            res = run_bass_kernel_spmd(nc, in_maps, core_ids=list(range(NCORES)))
            break
        except Exception as e:  # rare transient device-unrecoverable flakes
            last_err = e
            import time as _time

            _time.sleep(2.0)
    else:
        raise last_err
    psk = _combine(res.results, meta)
    loss = _hmm_tail(psk, tgt_idx, states, init_logps, trans_logps, ext_logps, hsmm_sid)
    return np.float32(loss)
